# revision 3
# baseline (speedup 1.0000x reference)
"""Trainium2 Bass kernel for single-head attention with input projections.

Problem: query (L=1024, N=16, E=1024), key/value (S=1024, N=16, E=1024),
q/k/v projection weights (E, E), in_proj_bias (3E,).
  q = (query @ Wq.T + bq) * E**-0.5
  k = key @ Wk.T + bk ; v = value @ Wv.T + bv
  out[l,n,f] = softmax_s(q[l,n,:] . k[s,n,:]) @ v[s,n,f]

Strategy: data-parallel over batch N across 8 NeuronCores (2 batches/core).

Algebraic restructuring (4 GEMMs/batch instead of 5): softmax over s is
invariant to per-l constants, so with G = Wq.T @ Wk * E^-0.5 (host-folded)
  scores[l,s] (mod softmax) = query[l] @ G @ key[s] + w[s]
  w[s] = key[s] . h,   h = (bq @ Wk) * E^-0.5  (host-folded)
and w folds in exactly via scores = key.T @ (qg + h x 1): add h per-partition
to the qg epilogue, like a bias. The k-projection GEMM disappears; bk and the
per-l bias terms drop out of the softmax. On the value side,
  out = (attn @ value) @ Wv.T + bv     (sum_s attn = 1 after normalization)
so the v-projection merges into the output path.

Device pipeline per batch (all dense bf16 matmuls, fp32 PSUM):
  qt  = G.T @ queryT + h        [f, l]   (G1)
  psT = xk.T @ qt ; es = exp(psT)  [s, l]   (G2; xk = raw keyT)
  ovT = xv.T @ es               [e, l]   (G3; xv = raw value rows [s, e])
  out = (ovT.T @ WvT) / rowsum + bv -> DRAM [l, f]   (G4)
rowsum via es_sum (DVE) + per-l-tile N=1 matmuls against ones (as before).

Post-compile, redundant LDWEIGHTS (same stationary operand as the previous
load) are deleted with their waits migrated to the following matmul; this
shrinks the PE instruction stream (fewer instruction-page fetch stalls).
"""

from contextlib import ExitStack

import numpy as np
import ml_dtypes

import concourse.bass as bass
import concourse.mybir as mybir
import concourse.tile as tile
from concourse import bacc
from concourse import bass_utils

L = 1024
S = 1024
E = 1024
N = 16
NCORES = 8
B = N // NCORES   # batches per core
P = 128
NF = 512          # psum free width (one fp32 bank)
KC = E // P
FT = E // P
LT = L // P
ST = S // P
LC = L // NF
FC = E // NF

BF = mybir.dt.bfloat16
F32 = mybir.dt.float32
AX = mybir.AluOpType
ACT_EXP = mybir.ActivationFunctionType.Exp
BF16 = ml_dtypes.bfloat16

_NC_CACHE = {}


def dedup_ldweights(nc):
    """Delete InstLdweights whose following Matmult uses the same stationary
    operand as the previous load; migrate any single wait to that matmul."""
    removed = 0
    for f in nc.m.functions:
        for blk in f.blocks:
            insts = blk.instructions
            last_key = None
            to_delete = []
            n = len(insts)
            for idx, ins in enumerate(insts):
                if not isinstance(ins, mybir.InstLdweights):
                    continue
                nxt = None
                for j in range(idx + 1, n):
                    if isinstance(insts[j], mybir.InstMatmult):
                        nxt = insts[j]
                        break
                    if isinstance(insts[j], mybir.InstLdweights):
                        break
                if nxt is None:
                    last_key = None
                    continue
                wap = nxt.ins[1]
                key = (wap.concise(), wap.offset)
                si = ins.sync_info
                waits = list(si.on_wait) if si is not None else []
                ups = list(si.on_update) if si is not None else []
                if key == last_key and not ups:
                    if not waits:
                        to_delete.append(ins)
                        removed += 1
                    else:
                        nsi = nxt.sync_info
                        nwaits = list(nsi.on_wait) if nsi is not None else []
                        if len(nwaits) + len(waits) <= 1:
                            nxt.sync_info = mybir.SyncInfo(
                                on_wait=waits + nwaits,
                                on_update=list(nsi.on_update) if nsi else [])
                            to_delete.append(ins)
                            removed += 1
                else:
                    last_key = key
            if to_delete:
                blk.instructions = [i for i in insts if i not in to_delete]
    return removed


def build_kernel():
    nc = bacc.Bacc("TRN2", target_bir_lowering=False, debug=False,
                   enable_asserts=False)

    qT_d = nc.declare_dram_parameter("qT", [B, E, L], BF, isOutput=False)
    kT_d = nc.declare_dram_parameter("kT", [B, E, S], BF, isOutput=False)
    vN_d = nc.declare_dram_parameter("vN", [B, S, E], BF, isOutput=False)
    g_d = nc.declare_dram_parameter("g", [E, E], BF, isOutput=False)
    wvT_d = nc.declare_dram_parameter("wvT", [E, E], BF, isOutput=False)
    h_d = nc.declare_dram_parameter("h", [P, FT], F32, isOutput=False)
    bv_d = nc.declare_dram_parameter("bv", [1, E], BF, isOutput=False)
    out_d = nc.declare_dram_parameter("out", [B, L, E], F32, isOutput=True)

    with tile.TileContext(nc) as tc, ExitStack() as ctx:
        sb = ctx.enter_context(tc.tile_pool(name="sb", bufs=1))
        dyn = ctx.enter_context(tc.tile_pool(name="dyn", bufs=2))
        wpool = apool = spool = sb
        opool = rpool = dyn
        psum = ctx.enter_context(
            tc.tile_pool(name="psum", bufs=8, space=bass.MemorySpace.PSUM))

        # ---- PE pre-warm first: memset + dummy matmuls keep the HAM
        # activity monitor busy so real matmuls start at full clock ----
        warm_sb = spool.tile([P, P], BF, tag="warm_sb")
        nc.vector.memset(warm_sb[:], 0.0)
        pwarm = psum.tile([P, P], F32, tag="mm", name="pwarm")
        for _ in range(14):
            nc.tensor.matmul(pwarm[:], warm_sb[:], warm_sb[:],
                             start=True, stop=True)

        # ---- persistent weights / constants ----
        g = wpool.tile([P, KC, E], BF, tag="g")
        wv = wpool.tile([P, KC, E], BF, tag="wv")
        h = spool.tile([P, FT], F32, tag="h")
        bv = spool.tile([1, E], BF, tag="bv")
        bvb = spool.tile([P, E], F32, tag="bvb")
        ones_r = spool.tile([1, P], BF, tag="ones_r")   # K=1 lhsT for bv bcast
        ones_c = spool.tile([P, 1], BF, tag="ones_c")   # N=1 rhs for rowsum
        nc.vector.memset(ones_r[:], 1.0)
        nc.vector.memset(ones_c[:], 1.0)

        for n in range(B):
            # ---- load activations ----
            xq = apool.tile([P, KC, L], BF, tag="xq")   # queryT [e, l]
            xk = apool.tile([P, KC, S], BF, tag="xk")   # keyT   [e, s]
            xv = apool.tile([P, KC, E], BF, tag="xv")   # value  [s, e]
            # batch 0: activations on the Scalar HWDGE queue in parallel with
            # weights on Sync; G1 consumes g[c]+xq[c] pairs as they land
            xeng = nc.scalar if n == 0 else nc.sync
            for c in range(KC):
                xeng.dma_start(out=xq[:, c, :], in_=qT_d[n, c * P:(c + 1) * P, :])
                if n == 0:
                    nc.sync.dma_start(out=g[:, c, :], in_=g_d[c * P:(c + 1) * P, :])
            if n == 0:
                nc.sync.dma_start(out=h[:], in_=h_d[:])
                xeng.dma_start(out=bv[:], in_=bv_d[:])
            for c in range(KC):
                xeng.dma_start(out=xk[:, c, :], in_=kT_d[n, c * P:(c + 1) * P, :])
            for c in range(KC):
                xeng.dma_start(out=xv[:, c, :], in_=vN_d[n, c * P:(c + 1) * P, :])
                if n == 0:
                    nc.sync.dma_start(out=wv[:, c, :], in_=wvT_d[c * P:(c + 1) * P, :])

            qt = apool.tile([P, FT, L], BF, tag="qt")   # qg.T + h  [f, l]
            es = apool.tile([P, ST, L], BF, tag="es")   # exp(scores.T) [s, l]
            ov = apool.tile([P, FT, L], BF, tag="ov")   # (attn @ value).T [e, l]
            es_sum = apool.tile([P, L], BF, tag="es_sum")

            # ---- G1: qt = G.T @ queryT + h; 3-3-2 f_tiles per pass so
            # per-chunk demand matches HBM chunk arrival while g and xq
            # stream in concurrently ----
            for fts in ((0, 1, 2), (3, 4, 5), (6, 7)):
                pq = [psum.tile([P, NF], F32, tag="mm", name="mm")
                      for _ in range(len(fts) * LC)]
                for c in range(KC):
                    for j, ft in enumerate(fts):
                        lhs = g[:, c, ft * P:(ft + 1) * P]
                        for lc in range(LC):
                            nc.tensor.matmul(pq[2 * j + lc][:], lhs,
                                             xq[:, c, lc * NF:(lc + 1) * NF],
                                             start=(c == 0), stop=(c == KC - 1))
                for j, ft in enumerate(fts):
                    for lc in range(LC):
                        nc.vector.tensor_scalar(
                            qt[:, ft, lc * NF:(lc + 1) * NF], pq[2 * j + lc][:],
                            h[:, ft:ft + 1], None, AX.add)
            if n == 0:
                # broadcast bv across partitions once: ones[1,128].T @ bv[1,:]
                for fc in range(FC):
                    pb = psum.tile([P, NF], F32, tag="mm", name="pb")
                    nc.tensor.matmul(pb[:], ones_r[:], bv[:, fc * NF:(fc + 1) * NF],
                                     start=True, stop=True)
                    nc.vector.tensor_copy(bvb[:, fc * NF:(fc + 1) * NF], pb[:])

            # ---- G2: scores.T = keyT.T @ qt, then exp ----
            for st in range(ST):
                ps = [psum.tile([P, NF], F32, tag="mm", name="mm") for _ in range(LC)]
                for c in range(KC):
                    lhs = xk[:, c, st * P:(st + 1) * P]
                    for lc in range(LC):
                        nc.tensor.matmul(ps[lc][:], lhs, qt[:, c, lc * NF:(lc + 1) * NF],
                                         start=(c == 0), stop=(c == KC - 1))
                for lc in range(LC):
                    nc.scalar.activation(es[:, st, lc * NF:(lc + 1) * NF],
                                         ps[lc][:], ACT_EXP)
                if st == 0:
                    nc.vector.tensor_copy(es_sum[:], es[:, 0, :])
                else:
                    nc.vector.tensor_tensor(es_sum[:], es_sum[:], es[:, st, :],
                                            AX.add)

            # ---- G3: ovT = value.T @ es ----
            for et in range(FT):
                pv = [psum.tile([P, NF], F32, tag="mm", name="mm") for _ in range(LC)]
                for c in range(ST):
                    lhs = xv[:, c, et * P:(et + 1) * P]
                    for lc in range(LC):
                        nc.tensor.matmul(pv[lc][:], lhs, es[:, c, lc * NF:(lc + 1) * NF],
                                         start=(c == 0), stop=(c == ST - 1))
                for lc in range(LC):
                    nc.vector.tensor_copy(ov[:, et, lc * NF:(lc + 1) * NF], pv[lc][:])

            # ---- G4: out = (ovT.T @ WvT) / rowsum + bv; store [l, f] ----
            pr8 = psum.tile([P, ST], F32, tag="mm", name="pr8")
            for lt in range(LT):
                po = [psum.tile([P, NF], F32, tag="mm", name="mm") for _ in range(FC)]
                recip = rpool.tile([P, 1], F32, tag="recip")
                ot = opool.tile([P, E], F32, tag="ot")
                if n == B - 1 and lt == LT - 1:
                    # final group: 4 half-width passes so each epilogue+store
                    # overlaps the next pass's matmuls; the last store splits
                    # across two HWDGE queues
                    nc.vector.reciprocal(recip[:], pr8[:, lt:lt + 1])
                    HF = NF // 2
                    for hp in range(2 * FC):
                        a = hp * HF
                        for c in range(FT):
                            nc.tensor.matmul(po[hp // 2][:, (hp % 2) * HF:(hp % 2 + 1) * HF],
                                             ov[:, c, lt * P:(lt + 1) * P],
                                             wv[:, c, a:a + HF],
                                             start=(c == 0), stop=(c == FT - 1))
                        nc.vector.scalar_tensor_tensor(
                            ot[:, a:a + HF],
                            po[hp // 2][:, (hp % 2) * HF:(hp % 2 + 1) * HF],
                            recip[:], bvb[:, a:a + HF], AX.mult, AX.add)
                        eng = nc.sync if hp % 2 == 0 else nc.scalar
                        eng.dma_start(
                            out=out_d[n, lt * P:(lt + 1) * P, a:a + HF],
                            in_=ot[:, a:a + HF])
                    continue
                for c in range(FT):
                    lhs = ov[:, c, lt * P:(lt + 1) * P]
                    for fc in range(FC):
                        nc.tensor.matmul(po[fc][:], lhs, wv[:, c, fc * NF:(fc + 1) * NF],
                                         start=(c == 0), stop=(c == FT - 1))
                    if lt == 0:
                        # per-l_tile rowsum: one N=1 matmul against the
                        # pre-summed exp tile
                        nc.tensor.matmul(pr8[:, c:c + 1],
                                         es_sum[:, c * P:(c + 1) * P], ones_c[:],
                                         start=True, stop=True)
                nc.vector.reciprocal(recip[:], pr8[:, lt:lt + 1])
                for fc in range(FC):
                    nc.vector.scalar_tensor_tensor(
                        ot[:, fc * NF:(fc + 1) * NF], po[fc][:],
                        recip[:], bvb[:, fc * NF:(fc + 1) * NF], AX.mult, AX.add)
                    nc.sync.dma_start(
                        out=out_d[n, lt * P:(lt + 1) * P, fc * NF:(fc + 1) * NF],
                        in_=ot[:, fc * NF:(fc + 1) * NF])

    nc.compile()
    dedup_ldweights(nc)
    return nc


def _get_nc():
    if "nc" not in _NC_CACHE:
        _NC_CACHE["nc"] = build_kernel()
    return _NC_CACHE["nc"]


def _make_in_maps(query, key, value, q_proj_weight, k_proj_weight,
                  v_proj_weight, in_proj_bias):
    q = np.asarray(query, np.float32)
    k = np.asarray(key, np.float32)
    v = np.asarray(value, np.float32)
    wq = np.asarray(q_proj_weight, np.float32)
    wk = np.asarray(k_proj_weight, np.float32)
    wv = np.asarray(v_proj_weight, np.float32)
    b = np.asarray(in_proj_bias, np.float32)
    scale = np.float32(E) ** -0.5

    # G = Wq.T @ Wk * scale  [e_in, f]; h = (bq @ Wk) * scale
    g = np.ascontiguousarray((wq.T @ wk) * scale).astype(BF16)
    hv = ((b[:E] @ wk) * scale).astype(np.float32)
    hs = np.ascontiguousarray(hv.reshape(FT, P).T)
    wvT = np.ascontiguousarray(wv.T).astype(BF16)
    bvs = b[2 * E:].astype(BF16).reshape(1, E)

    # (L, N, E) -> (N, E, L) for q/k; (S, N, E) -> (N, S, E) for v
    qT = np.ascontiguousarray(q.transpose(1, 2, 0)).astype(BF16)
    kT = np.ascontiguousarray(k.transpose(1, 2, 0)).astype(BF16)
    vN = np.ascontiguousarray(v.transpose(1, 0, 2)).astype(BF16)

    in_maps = []
    for i in range(NCORES):
        sl = slice(i * B, (i + 1) * B)
        in_maps.append({
            "qT": qT[sl], "kT": kT[sl], "vN": vN[sl],
            "g": g, "wvT": wvT, "h": hs, "bv": bvs,
        })
    return in_maps


def _run(inputs, trace=False, **kw):
    nc = _get_nc()
    in_maps = _make_in_maps(**inputs)
    res = bass_utils.run_bass_kernel_spmd(
        nc, in_maps, core_ids=list(range(NCORES)), trace=trace, **kw)
    # per-core out: (B, L, E) -> full (L, N, E)
    full = np.concatenate([res.results[i]["out"] for i in range(NCORES)], axis=0)
    out = np.ascontiguousarray(full.transpose(1, 0, 2))
    return out, res


def kernel(**inputs) -> np.ndarray:
    out, _ = _run(inputs, trace=False)
    return out
